# revision 21
# baseline (speedup 1.0000x reference)
"""Trainium2 Bass kernel for nn_MetaSREKPopulation (moe_routing).

Sharding: agent-parallel MLP phase (A=400 -> 50/core), AllToAll exchange of
normalized features, batch-parallel attention phase (B=64 -> 8/core),
AllToAll return of attention output, agent-parallel output heads.

All LayerNorm affine params and biases are folded into weight matrices
host-side (exact math): input-LN gain/bias fold into W1/Ws1; every GEMM bias
rides as an extra ones-row contraction; the post-attention aln LN affine folds
into Wq/Wk/Wv; Wo and the output head fold into a single per-agent WoWout.
"""
import sys
sys.path.insert(0, "/opt/trn_rl_repo")
import numpy as np

import concourse.bass as bass
import concourse.bacc as bacc
import concourse.mybir as mybir
import concourse.tile as tile
from concourse.bass_utils import run_bass_kernel_spmd
from concourse.masks import make_identity

F32 = mybir.dt.float32
AF = mybir.ActivationFunctionType
ALU = mybir.AluOpType
AX = mybir.AxisListType

A, B, I, H, O = 400, 64, 50, 192, 3
ADIM, NH, HD = 128, 8, 16
SCALE = HD ** -0.5
EPS = 1e-5
NC = 8           # cores
AL = A // NC     # agents per core = 50
RL = B // NC     # batch rows per core = 8
G = 8            # agents per MLP group
GROUPS = [list(range(s, min(s + G, AL))) for s in range(0, AL, G)]
HB = H - 128     # 64: second feature block
NCB = (A + 127) // 128  # 4 key blocks (128,128,128,16)
AR = A * RL
# head spreading: head h -> (tile t=h//3, strip 32*(h%3))
HT = [(h // 3, 32 * (h % 3)) for h in range(NH)]
NQT = 3          # number of spread q/k tiles

_cache = {}


def _prep(inputs):
    f = lambda t: np.asarray(t, np.float32)
    x = f(inputs["x"])
    W1, b1 = f(inputs["W1"]), f(inputs["b1"])
    Ws1, bs1 = f(inputs["Ws1"]), f(inputs["bs1"])
    W2, b2 = f(inputs["W2"]), f(inputs["b2"])
    Ws2, bs2 = f(inputs["Ws2"]), f(inputs["bs2"])
    W3, b3 = f(inputs["W3"]), f(inputs["b3"])
    gin, bin_ = f(inputs["ln_in_g"]), f(inputs["ln_in_b"])
    gh, bh = f(inputs["ln_h_g"]), f(inputs["ln_h_b"])
    Wout, bout = f(inputs["Wout"]), f(inputs["bout"])
    ag, ab = f(inputs["aln_g"]), f(inputs["aln_b"])
    Wq, bq = f(inputs["Wq"]), f(inputs["bq"])
    Wk, bk = f(inputs["Wk"]), f(inputs["bk"])
    Wv, bv = f(inputs["Wv"]), f(inputs["bv"])
    Wo, bo = f(inputs["Wo"]), f(inputs["bo"])

    W1p = np.concatenate([gin[:, :, None] * W1,
                          (np.einsum("ai,aih->ah", bin_, W1) + b1)[:, None, :]], 1)
    Ws1p = np.concatenate([gin[:, :, None] * Ws1,
                           (np.einsum("ai,aih->ah", bin_, Ws1) + bs1)[:, None, :]], 1)
    W2p = np.concatenate([W2, b2[:, None, :]], 1)
    Ws2p = np.concatenate([Ws2, bs2[:, None, :]], 1)
    W3p = np.concatenate([W3, b3[:, None, :]], 1)

    Wqp = np.concatenate([ag[:, None] * Wq, (ab @ Wq + bq)[None]], 0)  # [193,128]
    Wkp = np.concatenate([ag[:, None] * Wk, (ab @ Wk + bk)[None]], 0)
    Wvp = np.concatenate([ag[:, None] * Wv, (ab @ Wv + bv)[None]], 0)
    # spread q/k head columns onto 32-aligned strips: 3 tiles x <=3 heads
    WqpS = np.zeros((H + 1, NQT, 128), np.float32)
    WkpS = np.zeros((H + 1, NQT, 128), np.float32)
    for h in range(NH):
        t, p = HT[h]
        WqpS[:, t, p:p + HD] = Wqp[:, HD * h:HD * (h + 1)]
        WkpS[:, t, p:p + HD] = Wkp[:, HD * h:HD * (h + 1)]

    WoWout = np.einsum("dh,aho->ado", Wo, Wout)           # [A, 128, 3]
    cst = bout + bo @ Wout                                # [A, 3]
    Woutp = np.concatenate([Wout, cst[:, None, :]], 1)    # [A, 193, 3]

    # selector: BC col 32j+d reads recd row 32j+16
    seld = np.zeros((128, 128), np.float32)
    for j in range(4):
        seld[32 * j, 32 * j:32 * j + HD + 1] = 1.0

    in_maps = []
    for c in range(NC):
        sl = slice(c * AL, (c + 1) * AL)
        m = {
            "x": x,
            "w1p": np.ascontiguousarray(W1p[sl]),
            "ws1p": np.ascontiguousarray(Ws1p[sl]),
            "w2p": np.ascontiguousarray(W2p[sl]),
            "ws2p": np.ascontiguousarray(Ws2p[sl]),
            "w3p": np.ascontiguousarray(W3p[sl]),
            "ghT": np.ascontiguousarray(gh[sl].T),
            "bhT": np.ascontiguousarray(bh[sl].T),
            "woutT": np.ascontiguousarray(
                Woutp[sl].transpose(1, 0, 2).reshape(H + 1, AL * O)),
            "wwT": np.ascontiguousarray(
                WoWout[sl].transpose(1, 0, 2).reshape(ADIM, AL * O)),
            "wqps": WqpS.reshape(H + 1, NQT * 128),
            "wkps": WkpS.reshape(H + 1, NQT * 128),
            "wvp": Wvp,
            "seld": seld,
        }
        in_maps.append(m)
    return in_maps


def _build():
    nc = bacc.Bacc("TRN2", target_bir_lowering=False, debug=False,
                   num_devices=NC)
    d = {}
    def din(name, shape):
        d[name] = nc.dram_tensor(name, shape, F32, kind="ExternalInput")
    din("x", [B, I])
    din("w1p", [AL, I + 1, H]); din("ws1p", [AL, I + 1, H])
    din("w2p", [AL, H + 1, H]); din("ws2p", [AL, H + 1, H])
    din("w3p", [AL, H + 1, H])
    din("ghT", [H, AL]); din("bhT", [H, AL])
    din("woutT", [H + 1, AL * O]); din("wwT", [ADIM, AL * O])
    din("wqps", [H + 1, NQT * 128]); din("wkps", [H + 1, NQT * 128])
    din("wvp", [H + 1, ADIM])
    din("seld", [128, 128])
    out = nc.dram_tensor("logits", [AL, O, B], F32, kind="ExternalOutput")
    p1d = nc.dram_tensor("p1d", [AL, O, B], F32, kind="Internal")
    y_in = nc.dram_tensor("y_in", [NC, AL, H, RL], F32, kind="Internal")
    y_out = nc.dram_tensor("y_out", [NC, AL, H, RL], F32, kind="Internal")
    o_in = nc.dram_tensor("o_in", [NC, RL, ADIM, AL], F32, kind="Internal")
    o_out = nc.dram_tensor("o_out", [NC, RL, ADIM, AL], F32, kind="Internal")

    with tile.TileContext(nc) as tc:
        _emit(nc, tc, d, out, p1d, y_in, y_out, o_in, o_out)
    nc.compile()
    return nc


def _emit(nc, tc, d, out, p1d, y_in, y_out, o_in, o_out):
    with tc.tile_pool(name="const", bufs=1) as cpool, \
         tc.tile_pool(name="sb", bufs=2) as spool:
        _emit_body(nc, tc, d, out, p1d, y_in, y_out, o_in, o_out, cpool, spool)


def _emit_body(nc, tc, d, out, p1d, y_in, y_out, o_in, o_out, cpool, spool):
    # ---- constants ----
    ones_col = cpool.tile([128, 1], F32)
    nc.vector.memset(ones_col[:], 1.0)
    epsc = cpool.tile([128, 1], F32)
    nc.vector.memset(epsc[:], EPS)
    ones_row = cpool.tile([1, 128], F32)
    nc.vector.memset(ones_row[:], 1.0)
    ghT0 = cpool.tile([128, AL], F32)
    nc.sync.dma_start(out=ghT0[:], in_=d["ghT"].ap()[0:128, :])
    ghT1 = cpool.tile([HB, AL], F32)
    nc.sync.dma_start(out=ghT1[:], in_=d["ghT"].ap()[128:H, :])
    bhT0 = cpool.tile([128, AL], F32)
    nc.sync.dma_start(out=bhT0[:], in_=d["bhT"].ap()[0:128, :])
    bhT1 = cpool.tile([HB, AL], F32)
    nc.sync.dma_start(out=bhT1[:], in_=d["bhT"].ap()[128:H, :])
    woutT0 = cpool.tile([128, AL * O], F32)
    nc.sync.dma_start(out=woutT0[:], in_=d["woutT"].ap()[0:128, :])
    woutT1 = cpool.tile([H + 1 - 128, AL * O], F32)
    nc.sync.dma_start(out=woutT1[:], in_=d["woutT"].ap()[128:H + 1, :])

    # ---- phases 0+1 (phase-scoped pools) ----
    with tc.tile_pool(name="wts", bufs=3) as wpool, \
         tc.tile_pool(name="mlp", bufs=2) as hpool, \
         tc.tile_pool(name="pp1", bufs=2, space="PSUM") as ppool, \
         tc.tile_pool(name="pst", bufs=1, space="PSUM") as stpool:

        # xhat'^T [51, 64] shared by all agents
        ident = cpool.tile([128, 128], F32)
        make_identity(nc, ident)
        xt = spool.tile([B, I], F32, tag="ph0")
        nc.sync.dma_start(out=xt[:], in_=d["x"].ap())
        xstat = spool.tile([B, 1], F32, tag="ph0s")
        nc.vector.tensor_reduce(out=xstat[:], in_=xt[:], axis=AX.X, op=ALU.add)
        nc.vector.tensor_scalar_mul(xstat[:], xstat[:], 1.0 / I)
        xc = spool.tile([B, I], F32, tag="ph0c")
        nc.vector.tensor_scalar_sub(xc[:], xt[:], xstat[:])
        xsq = spool.tile([B, I], F32, tag="ph0q")
        nc.vector.tensor_tensor(out=xsq[:], in0=xc[:], in1=xc[:], op=ALU.mult)
        vstat = spool.tile([B, 1], F32, tag="ph0v")
        nc.vector.tensor_reduce(out=vstat[:], in_=xsq[:], axis=AX.X, op=ALU.add)
        sstat = spool.tile([B, 1], F32, tag="ph0sd")
        nc.scalar.activation(sstat[:], vstat[:], AF.Sqrt, bias=epsc[0:B, :],
                             scale=1.0 / I)
        rstat = spool.tile([B, 1], F32, tag="ph0r")
        nc.vector.reciprocal(rstat[:], sstat[:])
        nc.vector.tensor_scalar(out=xc[:], in0=xc[:], scalar1=rstat[:],
                                scalar2=None, op0=ALU.mult)
        xtp = stpool.tile([I, B], F32, tag="xT")
        nc.tensor.transpose(xtp[:], xc[:], ident[0:B, 0:B])
        XT = cpool.tile([I + 1, B], F32)
        nc.vector.memset(XT[:], 1.0)
        nc.scalar.copy(XT[0:I, :], xtp[:])

        for grp in GROUPS:
            _mlp_group(nc, tc, d, grp, cpool, spool, wpool, hpool, ppool,
                       stpool, XT, ghT0, ghT1, bhT0, bhT1, woutT0, woutT1,
                       ones_col, epsc, ones_row, p1d, y_in)

    nc.gpsimd.collective_compute(
        "AllToAll", ALU.bypass, replica_groups=[list(range(NC))],
        ins=[y_in.ap()], outs=[y_out.ap()])

    _attention(nc, tc, d, cpool, spool, y_out, o_in)

    nc.gpsimd.collective_compute(
        "AllToAll", ALU.bypass, replica_groups=[list(range(NC))],
        ins=[o_in.ap()], outs=[o_out.ap()])

    # ---- phase 3: logits = part1 + o @ WoWout ----
    with tc.tile_pool(name="ph3", bufs=3) as f3pool, \
         tc.tile_pool(name="pp3", bufs=2, space="PSUM") as p3pool:
        wwT = cpool.tile([ADIM, AL * O], F32)
        nc.sync.dma_start(out=wwT[:], in_=d["wwT"].ap())
        for grp in GROUPS:
            g0, gn = grp[0], len(grp)
            gb = gn * B
            p2 = p3pool.tile([O, G * B], F32, tag="p2")
            otg = f3pool.tile([ADIM, NC * RL * G], F32, tag="oin")
            for src in range(NC):
                nc.sync.dma_start(
                    out=otg[:, 64 * src:64 * (src + 1)].rearrange(
                        "dd (r a) -> dd r a", a=G)[:, :, 0:gn],
                    in_=o_out.ap()[src].rearrange("r dd a -> dd r a")[
                        :, :, g0:g0 + gn])
            for j, a in enumerate(grp):
                rhs = otg[:].rearrange("dd (src r a) -> dd src r a",
                                       r=RL, a=G)[:, :, :, j]
                nc.tensor.matmul(p2[:, j * B:(j + 1) * B],
                                 wwT[:, a * O:(a + 1) * O], rhs,
                                 start=True, stop=True)
            p1t = f3pool.tile([O, G * B], F32, tag="p1in")
            nc.sync.dma_start(
                out=p1t[:, 0:gb].rearrange("o (j b) -> o j b", b=B),
                in_=p1d.ap().rearrange("a o b -> o a b")[:, g0:g0 + gn, :])
            lg = f3pool.tile([O, G * B], F32, tag="lg")
            nc.vector.tensor_tensor(out=lg[:, 0:gb], in0=p2[:, 0:gb],
                                    in1=p1t[:, 0:gb], op=ALU.add)
            nc.sync.dma_start(
                out=out.ap().rearrange("a o b -> o a b")[:, g0:g0 + gn, :],
                in_=lg[:, 0:gb].rearrange("o (j b) -> o j b", b=B))


def _mlp_group(nc, tc, d, grp, cpool, spool, wpool, hpool, ppool, stpool,
               XT, ghT0, ghT1, bhT0, bhT1, woutT0, woutT1, ones_col, epsc,
               ones_row, p1d, y_in):
    g0, gn = grp[0], len(grp)
    gb = gn * B
    P1a = ppool.tile([128, G * B], F32, tag="mmA")
    P1b = ppool.tile([HB, G * B], F32, tag="mmB")
    P2a = ppool.tile([128, G * B], F32, tag="mmA")
    P2b = ppool.tile([HB, G * B], F32, tag="mmB")
    w1t = [wpool.tile([I + 1, H], F32, tag="w1", name=f"w1t{j}")
           for j in range(gn)]
    ws1t = [wpool.tile([I + 1, H], F32, tag="w1s", name=f"ws1t{j}")
            for j in range(gn)]
    for j, a in enumerate(grp):
        nc.sync.dma_start(out=w1t[j][:], in_=d["w1p"].ap()[a])
        nc.sync.dma_start(out=ws1t[j][:], in_=d["ws1p"].ap()[a])
    for j, a in enumerate(grp):
        ns = slice(j * B, (j + 1) * B)
        nc.tensor.matmul(P1a[:, ns], w1t[j][:, 0:128], XT[:], start=True, stop=True)
        nc.tensor.matmul(P1b[:, ns], w1t[j][:, 128:H], XT[:], start=True, stop=True)
        nc.tensor.matmul(P2a[:, ns], ws1t[j][:, 0:128], XT[:], start=True, stop=True)
        nc.tensor.matmul(P2b[:, ns], ws1t[j][:, 128:H], XT[:], start=True, stop=True)
    H1a, H1b = _gelu_add(nc, hpool, P1a, P1b, P2a, P2b, gb, "h1")

    P3a, P3b, P4a, P4b = _layer2(nc, d, wpool, ppool, grp, H1a, H1b, gb)
    H2a, H2b = _gelu_add(nc, hpool, P3a, P3b, P4a, P4b, gb, "h2")

    P5a, P5b = _layer3(nc, d, wpool, ppool, grp, H2a, H2b, gb)
    Za = hpool.tile([128, gb], F32, tag="za")
    nc.vector.tensor_tensor(out=Za[:], in0=H2a[:, 0:gb], in1=P5a[:, 0:gb],
                            op=ALU.add)
    Zb = hpool.tile([HB, gb], F32, tag="zb")
    nc.vector.tensor_tensor(out=Zb[:], in0=H2b[0:HB, 0:gb], in1=P5b[:, 0:gb],
                            op=ALU.add)

    mean, rstd = _pstats(nc, stpool, spool, hpool, ones_col, epsc, Za, Zb,
                         gb, "z")
    ZHa = hpool.tile([128, gb], F32, tag="zha")
    ZHb = hpool.tile([HB, gb], F32, tag="zhb")
    _normalize(nc, ppool, spool, ones_row, Za, Zb, mean, rstd, ZHa, ZHb, gb)
    H3a = hpool.tile([128, gb], F32, tag="h3a")
    H3b = hpool.tile([HB + 1, gb], F32, tag="h3b")
    for j, a in enumerate(grp):
        ns = slice(j * B, (j + 1) * B)
        nc.vector.tensor_scalar(
            out=H3a[:, ns], in0=ZHa[:, ns],
            scalar1=ghT0[:, a:a + 1], scalar2=bhT0[:, a:a + 1],
            op0=ALU.mult, op1=ALU.add)
        nc.vector.tensor_scalar(
            out=H3b[0:HB, ns], in0=ZHb[:, ns],
            scalar1=ghT1[:, a:a + 1], scalar2=bhT1[:, a:a + 1],
            op0=ALU.mult, op1=ALU.add)
    nc.vector.memset(H3b[HB:HB + 1, :], 1.0)

    m2, r2 = _pstats(nc, stpool, spool, hpool, ones_col, epsc, H3a, H3b,
                     gb, "y")
    Ya = hpool.tile([128, gb], F32, tag="ya")
    Yb = hpool.tile([HB, gb], F32, tag="yb")
    _normalize(nc, ppool, spool, ones_row, H3a, H3b, m2, r2, Ya, Yb, gb)

    # part1 = h3' @ Wout'
    p1 = stpool.tile([O, gb], F32, tag="p1")
    for j, a in enumerate(grp):
        ns = slice(j * B, (j + 1) * B)
        osl = slice(a * O, (a + 1) * O)
        nc.tensor.matmul(p1[:, ns], woutT0[:, osl], H3a[:, ns],
                         start=True, stop=False)
        nc.tensor.matmul(p1[:, ns], woutT1[:, osl], H3b[:, ns],
                         start=False, stop=True)
    p1s = spool.tile([O, G * B], F32, tag="p1s")
    nc.scalar.copy(p1s[:, 0:gb], p1[:])
    nc.sync.dma_start(
        out=p1d.ap().rearrange("a o b -> o a b")[:, g0:g0 + gn, :],
        in_=p1s[:, 0:gb].rearrange("o (j b) -> o j b", b=B))

    # yhat -> A2A input [dest, a, feat, r]
    ydst = y_in.ap().rearrange("dest a f r -> f dest a r")
    for dest in range(NC):
        nc.sync.dma_start(
            out=ydst[0:128, dest, g0:g0 + gn, :],
            in_=Ya[:].rearrange("f (j dest r) -> f dest j r",
                                dest=NC, r=RL)[:, dest, :, :])
        nc.sync.dma_start(
            out=ydst[128:H, dest, g0:g0 + gn, :],
            in_=Yb[:].rearrange("f (j dest r) -> f dest j r",
                                dest=NC, r=RL)[:, dest, :, :])


def _gelu_add(nc, hpool, Pa, Pb, Qa, Qb, gb, tagp):
    Ta = hpool.tile([128, gb], F32, tag=tagp + "ta")
    nc.scalar.activation(Ta[:], Pa[:, 0:gb], AF.Gelu)
    Tb = hpool.tile([HB, gb], F32, tag=tagp + "tb")
    nc.scalar.activation(Tb[:], Pb[:, 0:gb], AF.Gelu)
    Ha = hpool.tile([128, gb], F32, tag=tagp + "a")
    nc.vector.tensor_tensor(out=Ha[:], in0=Ta[:], in1=Qa[:, 0:gb], op=ALU.add)
    Hb = hpool.tile([HB + 1, gb], F32, tag=tagp + "b")
    nc.vector.tensor_tensor(out=Hb[0:HB, :], in0=Tb[:], in1=Qb[:, 0:gb],
                            op=ALU.add)
    nc.vector.memset(Hb[HB:HB + 1, :], 1.0)
    return Ha, Hb


def _layer2(nc, d, wpool, ppool, grp, Ha, Hb, gb):
    P3a = ppool.tile([128, G * B], F32, tag="mmA")
    P3b = ppool.tile([HB, G * B], F32, tag="mmB")
    P4a = ppool.tile([128, G * B], F32, tag="mmA")
    P4b = ppool.tile([HB, G * B], F32, tag="mmB")
    for j, a in enumerate(grp):
        ns = slice(j * B, (j + 1) * B)
        for nm, Pa, Pb in (("w2p", P3a, P3b), ("ws2p", P4a, P4b)):
            wa = wpool.tile([128, H], F32, tag="wka")
            nc.sync.dma_start(out=wa[:], in_=d[nm].ap()[a][0:128, :])
            wb = wpool.tile([H + 1 - 128, H], F32, tag="wkb")
            nc.sync.dma_start(out=wb[:], in_=d[nm].ap()[a][128:H + 1, :])
            nc.tensor.matmul(Pa[:, ns], wa[:, 0:128], Ha[:, ns],
                             start=True, stop=False)
            nc.tensor.matmul(Pa[:, ns], wb[:, 0:128], Hb[:, ns],
                             start=False, stop=True)
            nc.tensor.matmul(Pb[:, ns], wa[:, 128:H], Ha[:, ns],
                             start=True, stop=False)
            nc.tensor.matmul(Pb[:, ns], wb[:, 128:H], Hb[:, ns],
                             start=False, stop=True)
    return P3a, P3b, P4a, P4b


def _layer3(nc, d, wpool, ppool, grp, Ha, Hb, gb):
    P5a = ppool.tile([128, G * B], F32, tag="mmA")
    P5b = ppool.tile([HB, G * B], F32, tag="mmB")
    for j, a in enumerate(grp):
        ns = slice(j * B, (j + 1) * B)
        wa = wpool.tile([128, H], F32, tag="wka")
        nc.sync.dma_start(out=wa[:], in_=d["w3p"].ap()[a][0:128, :])
        wb = wpool.tile([H + 1 - 128, H], F32, tag="wkb")
        nc.sync.dma_start(out=wb[:], in_=d["w3p"].ap()[a][128:H + 1, :])
        nc.tensor.matmul(P5a[:, ns], wa[:, 0:128], Ha[:, ns], start=True, stop=False)
        nc.tensor.matmul(P5a[:, ns], wb[:, 0:128], Hb[:, ns], start=False, stop=True)
        nc.tensor.matmul(P5b[:, ns], wa[:, 128:H], Ha[:, ns], start=True, stop=False)
        nc.tensor.matmul(P5b[:, ns], wb[:, 128:H], Hb[:, ns], start=False, stop=True)
    return P5a, P5b


def _pstats(nc, stpool, spool, hpool, ones_col, epsc, Ta, Tb, gb, tag):
    s1 = stpool.tile([1, gb], F32, tag="s1")
    nc.tensor.matmul(s1[:], ones_col[:, :], Ta[:, 0:gb], start=True, stop=False)
    nc.tensor.matmul(s1[:], ones_col[0:HB, :], Tb[0:HB, 0:gb], start=False,
                     stop=True)
    SQa = hpool.tile([128, gb], F32, tag="sqa")
    nc.vector.tensor_tensor(out=SQa[:], in0=Ta[:, 0:gb], in1=Ta[:, 0:gb],
                            op=ALU.mult)
    SQb = hpool.tile([HB, gb], F32, tag="sqb")
    nc.vector.tensor_tensor(out=SQb[:], in0=Tb[0:HB, 0:gb], in1=Tb[0:HB, 0:gb],
                            op=ALU.mult)
    s2 = stpool.tile([1, gb], F32, tag="s2")
    nc.tensor.matmul(s2[:], ones_col[:, :], SQa[:], start=True, stop=False)
    nc.tensor.matmul(s2[:], ones_col[0:HB, :], SQb[:], start=False, stop=True)
    mean = spool.tile([1, gb], F32, tag=tag + "mean")
    nc.vector.tensor_scalar_mul(mean[:], s1[:], 1.0 / H)
    msq = spool.tile([1, gb], F32, tag=tag + "msq")
    nc.vector.tensor_tensor(out=msq[:], in0=mean[:], in1=mean[:], op=ALU.mult)
    varr = spool.tile([1, gb], F32, tag=tag + "var")
    nc.vector.scalar_tensor_tensor(out=varr[:], in0=s2[:], scalar=1.0 / H,
                                   in1=msq[:], op0=ALU.mult, op1=ALU.subtract)
    sd = spool.tile([1, gb], F32, tag=tag + "sd")
    nc.scalar.activation(sd[:], varr[:], AF.Sqrt, bias=epsc[0:1, :], scale=1.0)
    rstd = spool.tile([1, gb], F32, tag=tag + "rstd")
    nc.vector.reciprocal(rstd[:], sd[:])
    return mean, rstd


def _normalize(nc, ppool, spool, ones_row, Ta, Tb, mean, rstd, Oa, Ob, gb):
    """O = (T - mean) * rstd with [1, gb] stats broadcast via PE rank-1."""
    meanbc = ppool.tile([128, G * B], F32, tag="mmA")
    nc.tensor.matmul(meanbc[:, 0:gb], ones_row[:], mean[:], start=True, stop=True)
    rstdbc = ppool.tile([128, G * B], F32, tag="mmA")
    nc.tensor.matmul(rstdbc[:, 0:gb], ones_row[:], rstd[:], start=True, stop=True)
    t = spool.tile([128, gb], F32, tag="nrmt")
    nc.vector.tensor_tensor(out=t[:], in0=Ta[:, 0:gb], in1=meanbc[:, 0:gb],
                            op=ALU.subtract)
    nc.vector.tensor_tensor(out=Oa[:], in0=t[:], in1=rstdbc[:, 0:gb],
                            op=ALU.mult)
    t2 = spool.tile([HB, gb], F32, tag="nrmt2")
    nc.vector.tensor_tensor(out=t2[:], in0=Tb[0:HB, 0:gb],
                            in1=meanbc[0:HB, 0:gb], op=ALU.subtract)
    nc.vector.tensor_tensor(out=Ob[:], in0=t2[:], in1=rstdbc[0:HB, 0:gb],
                            op=ALU.mult)


def _attention(nc, tc, d, cpool, spool, y_out, o_in):
    with tc.tile_pool(name="abig", bufs=1) as big, \
         tc.tile_pool(name="awts", bufs=1) as awp, \
         tc.tile_pool(name="aact", bufs=3) as apool, \
         tc.tile_pool(name="vnp", bufs=5) as vpool, \
         tc.tile_pool(name="psc", bufs=1, space="PSUM") as psc, \
         tc.tile_pool(name="pot", bufs=2, space="PSUM") as pot, \
         tc.tile_pool(name="pmm", bufs=1, space="PSUM") as pmm:

        seld = cpool.tile([128, 128], F32)
        nc.sync.dma_start(out=seld[:], in_=d["seld"].ap())

        YTa = big.tile([128, AR], F32)
        YTb = big.tile([H + 1 - 128, AR], F32)
        for src in range(NC):
            nc.sync.dma_start(
                out=YTa[:].rearrange("f (s a r) -> f s a r",
                                     s=NC, r=RL)[:, src, :, :],
                in_=y_out.ap().rearrange("s a f r -> f s a r")[0:128, src, :, :])
            nc.sync.dma_start(
                out=YTb[0:HB, :].rearrange("f (s a r) -> f s a r",
                                           s=NC, r=RL)[:, src, :, :],
                in_=y_out.ap().rearrange("s a f r -> f s a r")[128:H, src, :, :])
        nc.vector.memset(YTb[HB:HB + 1, :], 1.0)

        # q/k projections into spread layout: 3 tiles each
        qT = [big.tile([128, AR], F32, name=f"qT{t}") for t in range(NQT)]
        kT = [big.tile([128, AR], F32, name=f"kT{t}") for t in range(NQT)]
        for nm, dsts in (("wqps", qT), ("wkps", kT)):
            wa = awp.tile([128, NQT * 128], F32, tag="wpja", name=f"wa_{nm}")
            nc.sync.dma_start(out=wa[:], in_=d[nm].ap()[0:128, :])
            wb = awp.tile([H + 1 - 128, NQT * 128], F32, tag="wpjb",
                          name=f"wb_{nm}")
            nc.sync.dma_start(out=wb[:], in_=d[nm].ap()[128:H + 1, :])
            for t in range(NQT):
                for s in range(0, AR, 512):
                    e = min(s + 512, AR)
                    pj = pmm.tile([128, 512], F32, tag="mm")
                    nc.tensor.matmul(pj[:, 0:e - s], wa[:, 128 * t:128 * (t + 1)],
                                     YTa[:, s:e], start=True, stop=False)
                    nc.tensor.matmul(pj[:, 0:e - s], wb[:, 128 * t:128 * (t + 1)],
                                     YTb[:, s:e], start=False, stop=True)
                    nc.scalar.copy(dsts[t][:, s:e], pj[:, 0:e - s])

        # v projection directly into [c, d] layout per r
        wva = awp.tile([128, ADIM], F32, tag="wpva")
        nc.sync.dma_start(out=wva[:], in_=d["wvp"].ap()[0:128, :])
        wvb = awp.tile([H + 1 - 128, ADIM], F32, tag="wpvb")
        nc.sync.dma_start(out=wvb[:], in_=d["wvp"].ap()[128:H + 1, :])

        for r in range(RL):
            VN = []
            for cb in range(NCB):
                c0 = cb * 128
                cn = min(128, A - c0)
                pv = pmm.tile([128, ADIM], F32, tag="mm")
                lhs_a = YTa[:].rearrange("f (a r) -> f a r", r=RL)[:, c0:c0 + cn, r]
                lhs_b = YTb[:].rearrange("f (a r) -> f a r", r=RL)[:, c0:c0 + cn, r]
                nc.tensor.matmul(pv[0:cn, :], lhs_a, wva[:], start=True, stop=False)
                nc.tensor.matmul(pv[0:cn, :], lhs_b, wvb[:], start=False, stop=True)
                vt = vpool.tile([128, NH * (HD + 1)], F32, tag="vn",
                                name=f"vt{cb}")
                nc.scalar.copy(
                    vt[0:cn, :].rearrange("c (h e) -> c h e", h=NH)[:, :, 1:HD + 1],
                    pv[0:cn, :].rearrange("c (h e) -> c h e", h=NH))
                nc.vector.memset(
                    vt[:, :].rearrange("c (h e) -> c h e", h=NH)[:, :, 0:1],
                    1.0)
                VN.append(vt)

            OTs = {}
            recds = {}
            for gi, heads in enumerate(((0, 1, 2, 3), (4, 5, 6, 7))):
                OT = pot.tile([128, A], F32, tag="ot")
                for hh in heads:
                    t, p = HT[hh]
                    sp = psc.tile([128, 2048], F32, tag="sc")
                    for cb in range(NCB):
                        c0 = cb * 128
                        cn = min(128, A - c0)
                        lhs = kT[t][p:p + HD, :].rearrange(
                            "dd (a r) -> dd a r", r=RL)[:, c0:c0 + cn, r]
                        rhs = qT[t][p:p + HD, :].rearrange(
                            "dd (a r) -> dd a r", r=RL)[:, :, r]
                        nc.tensor.matmul(sp[0:cn, cb * 512:cb * 512 + A],
                                         lhs, rhs, start=True, stop=True)
                    pb = apool.tile([128, NCB * A], F32, tag="prob")
                    nc.scalar.activation(
                        pb[:].rearrange("c (cb a) -> c cb a", cb=NCB),
                        sp[:].rearrange("c (cb x) -> c cb x", cb=NCB)[:, :, 0:A],
                        AF.Exp, scale=SCALE)
                    j = hh % 4
                    for cb in range(NCB):
                        c0 = cb * 128
                        cn = min(128, A - c0)
                        nc.tensor.matmul(
                            OT[32 * j:32 * j + HD + 1, :],
                            VN[cb][0:cn, (HD + 1) * hh:(HD + 1) * (hh + 1)],
                            pb[0:cn, cb * A:(cb + 1) * A],
                            start=(cb == 0), stop=(cb == NCB - 1),
                            tile_position=(0, 32 * j))
                rtmp = apool.tile([128, A], F32, tag="rtmp")
                nc.vector.tensor_scalar_max(rtmp[:], OT[:], 1e-30)
                recd = apool.tile([128, A], F32, tag="recd")
                nc.vector.reciprocal(recd[:], rtmp[:])
                OTs[gi] = OT
                recds[gi] = recd
            for gi in (0, 1):
                BC = pmm.tile([128, A], F32, tag="mmbc")
                nc.tensor.matmul(BC[:], seld[:], recds[gi][:], start=True,
                                 stop=True)
                osb = apool.tile([128, A], F32, tag="osb")
                nc.scalar.copy(osb[:], OTs[gi][:])
                ON = apool.tile([128, A], F32, tag="on")
                nc.vector.tensor_tensor(out=ON[:], in0=osb[:], in1=BC[:],
                                        op=ALU.mult)
                dv = o_in.ap().rearrange("dest r dd a -> dd dest r a")
                for hq in range(4):
                    d0 = 64 * gi + 16 * hq
                    nc.sync.dma_start(
                        out=dv[d0:d0 + HD, :, r, :],
                        in_=ON[32 * hq + 1:32 * hq + 1 + HD, :].rearrange(
                            "pp (s a) -> pp s a", s=NC))


def kernel(**inputs):
    in_maps = _prep(inputs)
    if "nc" not in _cache:
        _cache["nc"] = _build()
    nc = _cache["nc"]
    res = run_bass_kernel_spmd(nc, in_maps, list(range(NC)), trace=False)
    parts = [res.results[c]["logits"] for c in range(NC)]  # [AL, O, B]
    full = np.concatenate(parts, axis=0)                   # [A, O, B]
    return np.ascontiguousarray(full.transpose(0, 2, 1))   # [A, B, O]
